# revision 15
# baseline (speedup 1.0000x reference)
"""Multi-head causal attention (B=2, S=2048, D=1024, H=16, dk=64) on 8 TRN2 NeuronCores.

Sharding (data + head parallel, per the problem's sharding hint):
  core c -> batch b = c//4, head group g = c%4 (heads 4g..4g+3, i.e. a 256-wide
  column slice of the Q/K/V projections and a 256-row slice of w_o).

Per-core pipeline (fp16 compute, fp32 accumulation in PSUM):
  - x_q/x_k/x_v are loaded fp32, cast to fp16 on GpSimd, restaged through a
    DRAM scratch, and read back feature-major ([d, s]) via XBAR DMA-transpose
    (fp32 cannot XBAR-transpose; fp16 can, and this removes all PE transpose
    matmuls and their PSUM copybacks). Same for the four weight slices.
  - Q^T,K^T projected feature-on-partition; V natural with a ones column per
    head (softmax denominators fall out of the PV matmul for free).
  - Transposed scores S^T[k,q] = K ap Q^T per head; fp16 matmuls, fp32 PSUM.
  - exp on ScalarE straight out of PSUM with the 1/sqrt(dk) scale fused
    (no max-subtraction: inputs are unit-scale gaussians -> scaled scores are
    ~N(0,1); |s|<~8 so exp/sums cannot overflow fp32 or fp16 storage).
  - Causal masking: off-diagonal k-blocks skipped, dead column ranges of
    diagonal tiles never computed, the 128x128 diagonal squares masked in
    place by GpSimd affine_select.
  - PV^T accumulates unnormalized attention output feature-major + per-query
    denominators; normalization multiplies by a reciprocal broadcast across
    partitions with a K=1 PE matmul (gpsimd partition_broadcast is a ~13us
    software daisy-chain; the PE does it in ~0.4us). b_v is added afterwards
    (softmax rows sum to 1, so attn@(V+1 b_v^T) = attn@V + b_v).
  - w_o row-parallel partial product -> fp32 partial output.
Host sums the 4 partials per batch and adds b_o (the unshard of a row-parallel
w_o).
"""
import numpy as np

import concourse.bass as bass
import concourse.tile as tile
from concourse import bacc, mybir
from concourse.bass_utils import run_bass_kernel_spmd

F32 = mybir.dt.float32
F16 = mybir.dt.float16
AF = mybir.ActivationFunctionType
OP = mybir.AluOpType

B, S, D = 2, 2048, 1024
H, DK = 16, 64
NCORES = 8
HPC = 4            # heads per core
EPC = HPC * DK     # 256: e-slice width per core
SB = S // 128      # 16 s-blocks
DC = D // 128      # 8 d-chunks
QT_TILES = S // 512  # 4 q-tiles


def build_kernel(iters: int = 1):
    """Build the per-core Bass program. All 8 cores run the same program on
    different data (inputs are pre-sliced per core by the host)."""
    nc = bacc.Bacc("TRN2", target_bir_lowering=False, debug=False, num_devices=NCORES)

    xq = nc.dram_tensor("xq", [S, D], F32, kind="ExternalInput").ap()
    xk = nc.dram_tensor("xk", [S, D], F32, kind="ExternalInput").ap()
    xv = nc.dram_tensor("xv", [S, D], F32, kind="ExternalInput").ap()
    wq = nc.dram_tensor("wq", [EPC, D], F32, kind="ExternalInput").ap()
    wk = nc.dram_tensor("wk", [EPC, D], F32, kind="ExternalInput").ap()
    wv = nc.dram_tensor("wv", [EPC, D], F32, kind="ExternalInput").ap()
    wo = nc.dram_tensor("wo", [D, EPC], F32, kind="ExternalInput").ap()  # w_o[:, dslice]
    bq = nc.dram_tensor("bq", [EPC], F32, kind="ExternalInput").ap()
    bk = nc.dram_tensor("bk", [EPC], F32, kind="ExternalInput").ap()
    bv = nc.dram_tensor("bv", [EPC], F32, kind="ExternalInput").ap()
    out = nc.dram_tensor("out", [S, D], F32, kind="ExternalOutput").ap()

    with tile.TileContext(nc) as tc:
        with (
            tc.tile_pool(name="const", bufs=1) as cpool,
            tc.tile_pool(name="wT", bufs=1) as wpool,
            tc.tile_pool(name="xT", bufs=1) as xpool,
            tc.tile_pool(name="proj", bufs=1) as projpool,
            tc.tile_pool(name="nat", bufs=3) as natpool,
            tc.tile_pool(name="pt", bufs=4) as ptpool,
            tc.tile_pool(name="small", bufs=3) as smallpool,
            tc.tile_pool(name="oout", bufs=3) as opool,
            tc.tile_pool(name="dram", bufs=1, space="DRAM") as dpool,
            tc.tile_pool(name="ps_p", bufs=2, space="PSUM") as ps_p,
            tc.tile_pool(name="ps_s", bufs=3, space="PSUM") as ps_s,
            tc.tile_pool(name="ps_pv", bufs=2, space="PSUM") as ps_pv,
        ):
            # constants (outside the timing loop)
            ones_f32 = cpool.tile([128, max(SB * HPC, DK)], F32, tag="ones_f32")
            nc.gpsimd.memset(ones_f32[:], 1.0)
            ones_col = cpool.tile([1, DK], F16, tag="ones_col")
            nc.vector.tensor_copy(ones_col[:], ones_f32[0:1, 0:DK])

            # persistent tiles
            QT = projpool.tile([128, 2, S], F16, tag="QT")
            KT = projpool.tile([128, 2, S], F16, tag="KT")
            Vaug = projpool.tile([128, SB, HPC, DK + 1], F16, tag="Vaug")
            nc.vector.tensor_copy(Vaug[:, :, :, DK], ones_f32[:, 0:SB * HPC].rearrange("p (a b) -> p a b", a=SB))
            AOT = projpool.tile([128, 2, S], F16, tag="AOT")

            def stage_f16(src_ap, n_rows):
                """fp32 DRAM -> fp16 DRAM scratch (cast on GpSimd)."""
                n_cols = src_ap.shape[1]
                scr = dpool.tile([n_rows, n_cols], F16, tag=f"scr_{n_rows}_{n_cols}",
                                 name=f"scr{stage_f16.n}")
                stage_f16.n += 1
                for rc in range(n_rows // 128):
                    nat = natpool.tile([128, D], F32, tag="nat")
                    nc.sync.dma_start(nat[:, :n_cols], src_ap[rc * 128:(rc + 1) * 128, :])
                    nath = natpool.tile([128, D], F16, tag="nath")
                    nc.gpsimd.tensor_copy(nath[:, :n_cols], nat[:, :n_cols])
                    nc.sync.dma_start(scr[rc * 128:(rc + 1) * 128, :], nath[:, :n_cols])
                return scr
            stage_f16.n = 0

            def body():
                # ---- stage weights to fp16, read back feature-major
                wqT = wpool.tile([128, DC, EPC], F16, tag="wqT")
                wkT = wpool.tile([128, DC, EPC], F16, tag="wkT")
                wvT = wpool.tile([128, DC, EPC], F16, tag="wvT")
                woT = wpool.tile([128, 2, D], F16, tag="woT")
                for w_ap, wT in ((wq, wqT), (wk, wkT), (wv, wvT)):
                    scr = stage_f16(w_ap, EPC)
                    for dc in range(DC):
                        nc.sync.dma_start_transpose(wT[:, dc, :], scr[:, dc * 128:(dc + 1) * 128])
                scr = stage_f16(wo, D)
                for ch in range(2):
                    nc.sync.dma_start_transpose(woT[:, ch, :], scr[:, ch * 128:(ch + 1) * 128])

                bqT = cpool.tile([128, 2], F32, tag="bqT")
                bkT = cpool.tile([128, 2], F32, tag="bkT")
                bvT = cpool.tile([128, 2], F32, tag="bvT")
                nc.sync.dma_start(bqT[:], bq.rearrange("(c p) -> p c", p=128))
                nc.sync.dma_start(bkT[:], bk.rearrange("(c p) -> p c", p=128))
                nc.sync.dma_start(bvT[:], bv.rearrange("(c p) -> p c", p=128))

                # ---- projections
                for x_ap, wT, bT, dstT in ((xq, wqT, bqT, QT), (xk, wkT, bkT, KT)):
                    scr = stage_f16(x_ap, S)
                    xT = xpool.tile([128, DC, S], F16, tag="xT")
                    for dc in range(DC):
                        nc.sync.dma_start_transpose(xT[:, dc, :], scr[:, dc * 128:(dc + 1) * 128])
                    # dstT[e, s] = sum_d wT[d, e] * xT[d, s]  (+ bias[e])
                    for ec in range(2):
                        for st in range(QT_TILES):
                            pp = ps_p.tile([128, 512], F32, tag="pps")
                            for dc in range(DC):
                                nc.tensor.matmul(
                                    pp[:],
                                    wT[:, dc, ec * 128:(ec + 1) * 128],
                                    xT[:, dc, st * 512:(st + 1) * 512],
                                    start=(dc == 0), stop=(dc == DC - 1),
                                )
                            nc.scalar.activation(
                                dstT[:, ec, st * 512:(st + 1) * 512], pp[:],
                                AF.Identity, bias=bT[:, ec:ec + 1],
                            )

                # V: natural layout [s, e] (b_v folded in after attention)
                scr = stage_f16(xv, S)
                xT = xpool.tile([128, DC, S], F16, tag="xT")
                for dc in range(DC):
                    nc.sync.dma_start_transpose(xT[:, dc, :], scr[:, dc * 128:(dc + 1) * 128])
                for sb in range(SB):
                    pp = ps_p.tile([128, 512], F32, tag="pps")
                    for dc in range(DC):
                        nc.tensor.matmul(
                            pp[:, :EPC],
                            xT[:, dc, sb * 128:(sb + 1) * 128],
                            wvT[:, dc, :],
                            start=(dc == 0), stop=(dc == DC - 1),
                        )
                    nc.vector.tensor_copy(
                        Vaug[:, sb, :, 0:DK],
                        pp[:, :EPC].rearrange("p (h e) -> p h e", h=HPC),
                    )

                # ---- attention (S^T layout); heads 2ch (base 0) and 2ch+1 (base 64)
                # interleave so their matmuls can overlap in distinct PE row groups
                for ch in range(2):
                    heads = (2 * ch, 2 * ch + 1)
                    for qt in range(QT_TILES):
                        nkb = 4 * (qt + 1)
                        pvps = {}
                        for h in heads:
                            pvps[h] = ps_pv.tile([128, 512], F32, tag="pvp", name=f"pvp_{ch}_{qt}_{h}")
                        for kb in range(nkb):
                            j = kb - 4 * qt  # >= 0 only on diagonal blocks
                            lo = 128 * j if j >= 0 else 0
                            for h in heads:
                                base = 64 * (h % 2)
                                sp = ps_s.tile([128, 512], F32, tag="sps")
                                nc.tensor.matmul(
                                    sp[:, lo:512],
                                    KT[base:base + 64, ch, kb * 128:(kb + 1) * 128],
                                    QT[base:base + 64, ch, qt * 512 + lo:(qt + 1) * 512],
                                    start=True, stop=True,
                                )
                                pt_ = ptpool.tile([128, 512], F16, tag="ptile")
                                nc.scalar.activation(
                                    pt_[:, lo:512], sp[:, lo:512], AF.Exp, scale=0.125,
                                )
                                if j >= 0:
                                    # zero the strictly-upper triangle of the
                                    # diagonal square: keep where (c - r) >= 0
                                    nc.gpsimd.affine_select(
                                        out=pt_[:, lo:lo + 128], in_=pt_[:, lo:lo + 128],
                                        compare_op=OP.is_ge, fill=0.0,
                                        base=0, pattern=[[1, 128]], channel_multiplier=-1,
                                    )
                                nc.tensor.matmul(
                                    pvps[h][0:DK + 1, lo:512],
                                    Vaug[:, kb, h, :],
                                    pt_[:, lo:512],
                                    start=(kb == 0), stop=(kb == nkb - 1),
                                )
                        for h in heads:
                            base = 64 * (h % 2)
                            pvp = pvps[h]
                            rec = smallpool.tile([1, 512], F16, tag="rec")
                            with nc.allow_low_precision(reason="softmax reciprocal in fp16; sums are O(1e3)"):
                                nc.vector.reciprocal(rec[:], pvp[DK:DK + 1, :])
                            # broadcast rec across 64 partitions via K=1 matmul
                            recp = ps_s.tile([128, 512], F32, tag="sps", name=f"recp_{ch}_{qt}_{h}")
                            nc.tensor.matmul(
                                recp[0:DK, :], ones_col[:], rec[:],
                                start=True, stop=True,
                            )
                            recb = smallpool.tile([64, 512], F32, tag="recb")
                            nc.vector.tensor_copy(recb[:], recp[0:DK, :])
                            aslice = AOT[base:base + 64, ch, qt * 512:(qt + 1) * 512]
                            nc.vector.tensor_tensor(aslice, pvp[0:DK, :], recb[:], OP.mult)
                            nc.gpsimd.tensor_tensor(
                                aslice, aslice,
                                bvT[base:base + 64, ch, None].to_broadcast((64, 512)),
                                OP.add,
                            )

                # ---- w_o partial: out[s, e] = sum_d AOT[d, s] * woT[d, e]
                for sb in range(SB):
                    for et in range(2):
                        pw = ps_p.tile([128, 512], F32, tag="pps")
                        for ch in range(2):
                            nc.tensor.matmul(
                                pw[:],
                                AOT[:, ch, sb * 128:(sb + 1) * 128],
                                woT[:, ch, et * 512:(et + 1) * 512],
                                start=(ch == 0), stop=(ch == 1),
                            )
                        ot = opool.tile([128, 512], F32, tag="otile")
                        nc.vector.tensor_copy(ot[:], pw[:])
                        nc.sync.dma_start(
                            out[sb * 128:(sb + 1) * 128, et * 512:(et + 1) * 512], ot[:],
                        )

            if iters == 1:
                body()
            else:
                with tc.For_i(0, iters, 1):
                    body()

    nc.compile()
    return nc


_NC_CACHE = {}


def _get_nc(iters: int = 1):
    if iters not in _NC_CACHE:
        _NC_CACHE[iters] = build_kernel(iters)
    return _NC_CACHE[iters]


def make_in_maps(query, key, value, w_q, b_q, w_k, b_k, w_v, b_v, w_o, b_o):
    in_maps = []
    for c in range(NCORES):
        b = c // 4
        g = c % 4
        es = slice(EPC * g, EPC * (g + 1))
        in_maps.append({
            "xq": np.ascontiguousarray(query[b], np.float32),
            "xk": np.ascontiguousarray(key[b], np.float32),
            "xv": np.ascontiguousarray(value[b], np.float32),
            "wq": np.ascontiguousarray(w_q[es, :], np.float32),
            "wk": np.ascontiguousarray(w_k[es, :], np.float32),
            "wv": np.ascontiguousarray(w_v[es, :], np.float32),
            "wo": np.ascontiguousarray(w_o[:, es], np.float32),
            "bq": np.ascontiguousarray(b_q[es], np.float32),
            "bk": np.ascontiguousarray(b_k[es], np.float32),
            "bv": np.ascontiguousarray(b_v[es], np.float32),
        })
    return in_maps


def kernel(query, key, value, w_q, b_q, w_k, b_k, w_v, b_v, w_o, b_o, _iters=1):
    query = np.asarray(query, np.float32)
    key = np.asarray(key, np.float32)
    value = np.asarray(value, np.float32)
    w_q, b_q = np.asarray(w_q, np.float32), np.asarray(b_q, np.float32)
    w_k, b_k = np.asarray(w_k, np.float32), np.asarray(b_k, np.float32)
    w_v, b_v = np.asarray(w_v, np.float32), np.asarray(b_v, np.float32)
    w_o, b_o = np.asarray(w_o, np.float32), np.asarray(b_o, np.float32)

    nc = _get_nc(_iters)
    in_maps = make_in_maps(query, key, value, w_q, b_q, w_k, b_k, w_v, b_v, w_o, b_o)
    res = run_bass_kernel_spmd(nc, in_maps, core_ids=list(range(NCORES)))

    # unshard: sum the 4 row-parallel partials per batch, add b_o
    full = np.empty((B, S, D), np.float32)
    for b in range(B):
        acc = res.results[4 * b]["out"].astype(np.float32)
        for g in range(1, 4):
            acc = acc + res.results[4 * b + g]["out"]
        full[b] = acc + b_o[None, :]
    return full


# revision 16
# speedup vs baseline: 1.3589x; 1.3589x over previous
"""Multi-head causal attention (B=2, S=2048, D=1024, H=16, dk=64) on 8 TRN2 NeuronCores.

Sharding (data + head parallel, per the problem's sharding hint):
  core c -> batch b = c//4, head group g = c%4 (heads 4g..4g+3, i.e. a 256-wide
  column slice of the Q/K/V projections and a 256-row slice of w_o).

Per-core pipeline (fp16 compute, fp32 accumulation in PSUM):
  - x_q/x_k/x_v tiles are loaded fp32 and cast to fp16 on GpSimd (idle engine),
    then PE-transposed to feature-major x^T (fp16 transposes run 2x faster than
    fp32 and their PSUM copybacks hit the DVE 2x mode); 4 transpose blocks are
    batched per PSUM bank with one strided copyback each.
  - Q^T,K^T projected feature-on-partition; V natural with a ones column per
    head (softmax denominators fall out of the PV matmul for free). PSUM
    accumulation chains are emitted pairwise-interleaved across two banks to
    hide the accumulate-to-same-bank latency.
  - Transposed scores S^T[k,q] = K ap Q^T per head; even/odd heads of a pair sit
    at partition bases 0/64 and issue back-to-back so the PE can overlap them
    in distinct row groups.
  - exp on ScalarE straight out of PSUM with the 1/sqrt(dk) scale fused
    (no max-subtraction: inputs are unit-scale gaussians -> scaled scores are
    ~N(0,1); |s|<~8 so exp/sums cannot overflow fp32 or fp16 storage).
  - Causal masking: off-diagonal k-blocks skipped, dead column ranges of
    diagonal tiles never computed, the 128x128 diagonal squares masked in
    place by GpSimd affine_select.
  - PV^T accumulates unnormalized attention output feature-major + per-query
    denominators; normalization multiplies by a reciprocal broadcast across
    partitions with a K=1 PE matmul (gpsimd partition_broadcast is a ~13us
    software daisy-chain; the PE does it in ~0.4us). b_v is added afterwards
    (softmax rows sum to 1, so attn@(V+1 b_v^T) = attn@V + b_v).
  - w_o row-parallel partial product -> fp32 partial output.
Host sums the 4 partials per batch and adds b_o (the unshard of a row-parallel
w_o).
"""
import numpy as np

import concourse.bass as bass
import concourse.tile as tile
from concourse import bacc, mybir
from concourse.bass_utils import run_bass_kernel_spmd
from concourse.masks import make_identity

F32 = mybir.dt.float32
F16 = mybir.dt.float16
AF = mybir.ActivationFunctionType
OP = mybir.AluOpType

B, S, D = 2, 2048, 1024
H, DK = 16, 64
NCORES = 8
HPC = 4            # heads per core
EPC = HPC * DK     # 256: e-slice width per core
SB = S // 128      # 16 s-blocks
DC = D // 128      # 8 d-chunks
QT_TILES = S // 512  # 4 q-tiles


def build_kernel(iters: int = 1):
    """Build the per-core Bass program. All 8 cores run the same program on
    different data (inputs are pre-sliced per core by the host)."""
    nc = bacc.Bacc("TRN2", target_bir_lowering=False, debug=False, num_devices=NCORES)

    xq = nc.dram_tensor("xq", [S, D], F32, kind="ExternalInput").ap()
    xk = nc.dram_tensor("xk", [S, D], F32, kind="ExternalInput").ap()
    xv = nc.dram_tensor("xv", [S, D], F32, kind="ExternalInput").ap()
    wq = nc.dram_tensor("wq", [EPC, D], F32, kind="ExternalInput").ap()
    wk = nc.dram_tensor("wk", [EPC, D], F32, kind="ExternalInput").ap()
    wv = nc.dram_tensor("wv", [EPC, D], F32, kind="ExternalInput").ap()
    wo = nc.dram_tensor("wo", [D, EPC], F32, kind="ExternalInput").ap()  # w_o[:, dslice]
    bq = nc.dram_tensor("bq", [EPC], F32, kind="ExternalInput").ap()
    bk = nc.dram_tensor("bk", [EPC], F32, kind="ExternalInput").ap()
    bv = nc.dram_tensor("bv", [EPC], F32, kind="ExternalInput").ap()
    out = nc.dram_tensor("out", [S, D], F32, kind="ExternalOutput").ap()

    with tile.TileContext(nc) as tc:
        with (
            tc.tile_pool(name="const", bufs=1) as cpool,
            tc.tile_pool(name="wT", bufs=1) as wpool,
            tc.tile_pool(name="xT", bufs=1) as xpool,
            tc.tile_pool(name="proj", bufs=1) as projpool,
            tc.tile_pool(name="nat", bufs=3) as natpool,
            tc.tile_pool(name="pt", bufs=6) as ptpool,
            tc.tile_pool(name="small", bufs=3) as smallpool,
            tc.tile_pool(name="oout", bufs=3) as opool,
            tc.tile_pool(name="ps_p", bufs=2, space="PSUM") as ps_p,
            tc.tile_pool(name="ps_s", bufs=3, space="PSUM") as ps_s,
            tc.tile_pool(name="ps_pv", bufs=3, space="PSUM") as ps_pv,
        ):
            # constants (outside the timing loop)
            ident = cpool.tile([128, 128], F16)
            make_identity(nc, ident)
            ones_f32 = cpool.tile([128, max(SB * HPC, DK)], F32, tag="ones_f32")
            nc.gpsimd.memset(ones_f32[:], 1.0)
            ones_col = cpool.tile([1, DK], F16, tag="ones_col")
            nc.vector.tensor_copy(ones_col[:], ones_f32[0:1, 0:DK])

            # persistent tiles
            QT = projpool.tile([128, 2, S], F16, tag="QT")
            KT = projpool.tile([128, 2, S], F16, tag="KT")
            Vaug = projpool.tile([128, SB, HPC, DK + 1], F16, tag="Vaug")
            nc.vector.tensor_copy(
                Vaug[:, :, :, DK],
                ones_f32[:, 0:SB * HPC].rearrange("p (a b) -> p a b", a=SB))
            AOT = projpool.tile([128, 2, S], F16, tag="AOT")

            copyback_flip = [0]

            def transpose_into(dst_idx, src_ap, n_row_tiles, n_col_chunks):
                """Load fp32 row-tiles, cast to fp16 on GpSimd, PE-transpose
                (4 blocks batched per PSUM bank), strided copyback alternating
                DVE/ACT. dst_idx(rc, cc0, nb) -> [128, nb, 128] destination."""
                n_cols = src_ap.shape[1]
                for rc in range(n_row_tiles):
                    nat = natpool.tile([128, D], F32, tag="nat")
                    nc.sync.dma_start(nat[:, :n_cols], src_ap[rc * 128:(rc + 1) * 128, :])
                    nath = natpool.tile([128, D], F16, tag="nath")
                    nc.gpsimd.tensor_copy(nath[:, :n_cols], nat[:, :n_cols])
                    cc0 = 0
                    while cc0 < n_col_chunks:
                        nb = min(4, n_col_chunks - cc0)
                        pt_ = ps_p.tile([128, 512], F16, tag="pps", name=f"tps{copyback_flip[0]}")
                        for j in range(nb):
                            nc.tensor.matmul(
                                pt_[:, j * 128:(j + 1) * 128],
                                nath[:, (cc0 + j) * 128:(cc0 + j + 1) * 128],
                                ident[:],
                                is_transpose=True, start=(j == 0), stop=(j == nb - 1),
                            )
                        src_view = pt_[:, :nb * 128].rearrange("p (b c) -> p b c", b=nb)
                        d_slice = dst_idx(rc, cc0, nb)
                        if copyback_flip[0] % 2 == 0:
                            nc.vector.tensor_copy(d_slice, src_view)
                        else:
                            nc.scalar.activation(d_slice, src_view, AF.Copy)
                        copyback_flip[0] += 1
                        cc0 += nb

            def body():
                # ---- weights: cast + transpose to feature-major
                wqT = wpool.tile([128, DC, EPC], F16, tag="wqT")
                wkT = wpool.tile([128, DC, EPC], F16, tag="wkT")
                wvT = wpool.tile([128, DC, EPC], F16, tag="wvT")
                woT = wpool.tile([128, 2, D], F16, tag="woT")
                for w_ap, wT in ((wq, wqT), (wk, wkT), (wv, wvT)):
                    transpose_into(
                        lambda rc, cc0, nb, wT=wT: wT[:, cc0:cc0 + nb, rc * 128:(rc + 1) * 128],
                        w_ap, EPC // 128, DC)
                transpose_into(
                    lambda rc, cc0, nb: woT[:, cc0:cc0 + nb, rc * 128:(rc + 1) * 128],
                    wo, DC, 2)

                bqT = cpool.tile([128, 2], F32, tag="bqT")
                bkT = cpool.tile([128, 2], F32, tag="bkT")
                bvT = cpool.tile([128, 2], F32, tag="bvT")
                nc.sync.dma_start(bqT[:], bq.rearrange("(c p) -> p c", p=128))
                nc.sync.dma_start(bkT[:], bk.rearrange("(c p) -> p c", p=128))
                nc.sync.dma_start(bvT[:], bv.rearrange("(c p) -> p c", p=128))

                # ---- projections (accumulation chains pairwise-interleaved)
                for x_ap, wT, bT, dstT in ((xq, wqT, bqT, QT), (xk, wkT, bkT, KT)):
                    xT = xpool.tile([128, DC, S], F16, tag="xT")
                    transpose_into(
                        lambda rc, cc0, nb, xT=xT: xT[:, cc0:cc0 + nb, rc * 128:(rc + 1) * 128],
                        x_ap, SB, DC)
                    # dstT[e, s] = sum_d wT[d, e] * xT[d, s]  (+ bias[e])
                    for ec in range(2):
                        for st0 in range(0, QT_TILES, 2):
                            pps = [ps_p.tile([128, 512], F32, tag="pps",
                                             name=f"pp_{ec}_{st0}_{k}") for k in range(2)]
                            for dc in range(DC):
                                for k in range(2):
                                    nc.tensor.matmul(
                                        pps[k][:],
                                        wT[:, dc, ec * 128:(ec + 1) * 128],
                                        xT[:, dc, (st0 + k) * 512:(st0 + k + 1) * 512],
                                        start=(dc == 0), stop=(dc == DC - 1),
                                    )
                            for k in range(2):
                                nc.scalar.activation(
                                    dstT[:, ec, (st0 + k) * 512:(st0 + k + 1) * 512], pps[k][:],
                                    AF.Identity, bias=bT[:, ec:ec + 1],
                                )

                # V: natural layout [s, e] (b_v folded in after attention)
                xT = xpool.tile([128, DC, S], F16, tag="xT")
                transpose_into(
                    lambda rc, cc0, nb, xT=xT: xT[:, cc0:cc0 + nb, rc * 128:(rc + 1) * 128],
                    xv, SB, DC)
                for sb0 in range(0, SB, 2):
                    pps = [ps_p.tile([128, 512], F32, tag="pps",
                                     name=f"ppv_{sb0}_{k}") for k in range(2)]
                    for dc in range(DC):
                        for k in range(2):
                            nc.tensor.matmul(
                                pps[k][:, :EPC],
                                xT[:, dc, (sb0 + k) * 128:(sb0 + k + 1) * 128],
                                wvT[:, dc, :],
                                start=(dc == 0), stop=(dc == DC - 1),
                            )
                    for k in range(2):
                        nc.vector.tensor_copy(
                            Vaug[:, sb0 + k, :, 0:DK],
                            pps[k][:, :EPC].rearrange("p (h e) -> p h e", h=HPC),
                        )

                # ---- attention (S^T layout); heads 2ch (base 0) and 2ch+1 (base 64)
                for ch in range(2):
                    heads = (2 * ch, 2 * ch + 1)
                    for qt in range(QT_TILES):
                        nkb = 4 * (qt + 1)
                        pvps = {}
                        for h in heads:
                            pvps[h] = ps_pv.tile([128, 512], F32, tag="pvp", name=f"pvp_{ch}_{qt}_{h}")
                        for kb in range(nkb):
                            j = kb - 4 * qt  # >= 0 only on diagonal blocks
                            lo = 128 * j if j >= 0 else 0
                            for h in heads:
                                base = 64 * (h % 2)
                                sp = ps_s.tile([128, 512], F32, tag="sps")
                                nc.tensor.matmul(
                                    sp[:, lo:512],
                                    KT[base:base + 64, ch, kb * 128:(kb + 1) * 128],
                                    QT[base:base + 64, ch, qt * 512 + lo:(qt + 1) * 512],
                                    start=True, stop=True,
                                )
                                pt_ = ptpool.tile([128, 512], F16, tag="ptile")
                                nc.scalar.activation(
                                    pt_[:, lo:512], sp[:, lo:512], AF.Exp, scale=0.125,
                                )
                                if j >= 0:
                                    # zero the strictly-upper triangle of the
                                    # diagonal square: keep where (c - r) >= 0
                                    nc.gpsimd.affine_select(
                                        out=pt_[:, lo:lo + 128], in_=pt_[:, lo:lo + 128],
                                        compare_op=OP.is_ge, fill=0.0,
                                        base=0, pattern=[[1, 128]], channel_multiplier=-1,
                                    )
                                nc.tensor.matmul(
                                    pvps[h][0:DK + 1, lo:512],
                                    Vaug[:, kb, h, :],
                                    pt_[:, lo:512],
                                    start=(kb == 0), stop=(kb == nkb - 1),
                                )
                        for h in heads:
                            base = 64 * (h % 2)
                            pvp = pvps[h]
                            rec = smallpool.tile([1, 512], F16, tag="rec")
                            with nc.allow_low_precision(reason="softmax reciprocal in fp16; sums are O(1e3)"):
                                nc.vector.reciprocal(rec[:], pvp[DK:DK + 1, :])
                            # broadcast rec across 64 partitions via K=1 matmul
                            recp = ps_s.tile([128, 512], F32, tag="sps", name=f"recp_{ch}_{qt}_{h}")
                            nc.tensor.matmul(
                                recp[0:DK, :], ones_col[:], rec[:],
                                start=True, stop=True,
                            )
                            recb = smallpool.tile([64, 512], F32, tag="recb")
                            nc.vector.tensor_copy(recb[:], recp[0:DK, :])
                            aslice = AOT[base:base + 64, ch, qt * 512:(qt + 1) * 512]
                            nc.vector.tensor_tensor(aslice, pvp[0:DK, :], recb[:], OP.mult)
                            nc.gpsimd.tensor_tensor(
                                aslice, aslice,
                                bvT[base:base + 64, ch, None].to_broadcast((64, 512)),
                                OP.add,
                            )

                # ---- w_o partial: out[s, e] = sum_d AOT[d, s] * woT[d, e]
                for sb in range(SB):
                    pws = [ps_p.tile([128, 512], F32, tag="pps",
                                     name=f"pw_{sb}_{et}") for et in range(2)]
                    for ch in range(2):
                        for et in range(2):
                            nc.tensor.matmul(
                                pws[et][:],
                                AOT[:, ch, sb * 128:(sb + 1) * 128],
                                woT[:, ch, et * 512:(et + 1) * 512],
                                start=(ch == 0), stop=(ch == 1),
                            )
                    for et in range(2):
                        ot = opool.tile([128, 512], F32, tag="otile")
                        nc.vector.tensor_copy(ot[:], pws[et][:])
                        nc.sync.dma_start(
                            out[sb * 128:(sb + 1) * 128, et * 512:(et + 1) * 512], ot[:],
                        )

            if iters == 1:
                body()
            else:
                with tc.For_i(0, iters, 1):
                    body()

    nc.compile()
    return nc


_NC_CACHE = {}


def _get_nc(iters: int = 1):
    if iters not in _NC_CACHE:
        _NC_CACHE[iters] = build_kernel(iters)
    return _NC_CACHE[iters]


def make_in_maps(query, key, value, w_q, b_q, w_k, b_k, w_v, b_v, w_o, b_o):
    in_maps = []
    for c in range(NCORES):
        b = c // 4
        g = c % 4
        es = slice(EPC * g, EPC * (g + 1))
        in_maps.append({
            "xq": np.ascontiguousarray(query[b], np.float32),
            "xk": np.ascontiguousarray(key[b], np.float32),
            "xv": np.ascontiguousarray(value[b], np.float32),
            "wq": np.ascontiguousarray(w_q[es, :], np.float32),
            "wk": np.ascontiguousarray(w_k[es, :], np.float32),
            "wv": np.ascontiguousarray(w_v[es, :], np.float32),
            "wo": np.ascontiguousarray(w_o[:, es], np.float32),
            "bq": np.ascontiguousarray(b_q[es], np.float32),
            "bk": np.ascontiguousarray(b_k[es], np.float32),
            "bv": np.ascontiguousarray(b_v[es], np.float32),
        })
    return in_maps


def kernel(query, key, value, w_q, b_q, w_k, b_k, w_v, b_v, w_o, b_o, _iters=1):
    query = np.asarray(query, np.float32)
    key = np.asarray(key, np.float32)
    value = np.asarray(value, np.float32)
    w_q, b_q = np.asarray(w_q, np.float32), np.asarray(b_q, np.float32)
    w_k, b_k = np.asarray(w_k, np.float32), np.asarray(b_k, np.float32)
    w_v, b_v = np.asarray(w_v, np.float32), np.asarray(b_v, np.float32)
    w_o, b_o = np.asarray(w_o, np.float32), np.asarray(b_o, np.float32)

    nc = _get_nc(_iters)
    in_maps = make_in_maps(query, key, value, w_q, b_q, w_k, b_k, w_v, b_v, w_o, b_o)
    res = run_bass_kernel_spmd(nc, in_maps, core_ids=list(range(NCORES)))

    # unshard: sum the 4 row-parallel partials per batch, add b_o
    full = np.empty((B, S, D), np.float32)
    for b in range(B):
        acc = res.results[4 * b]["out"].astype(np.float32)
        for g in range(1, 4):
            acc = acc + res.results[4 * b + g]["out"]
        full[b] = acc + b_o[None, :]
    return full
